# revision 22
# baseline (speedup 1.0000x reference)
"""Data-parallel Trainium2 kernel for the weighted classification loss.

loss = -mean_b sum_c w[b,c] * log(1 - softmax(reps @ W.T + b)[b,c])

Strategy (8 cores, batch-sharded 4096 rows each):
  - Host pre-transposes and casts reps to fp8 e4m3 (and W, scaled by 64)
    laid out so each DMA chunk is contiguous 2KB partition lines;
    quantization error on the final mean loss is ~4e-5 (verified vs f64
    on host).
  - fp8 DoubleRow matmuls (K=256/pass): 4 passes x 4 col-blocks per
    1024-col stage accumulate logits^T*64 into PSUM [10, 1024].
  - DVE repacks 4 col-blocks to partition groups 10g -> lg40 [40, 256]
    so the rest of the pipeline uses 40+ partitions.
  - exp(l/64 + b) on ACT; one K=40 matmul with a block-diag
    (ones-I | ones) stationary gives u = den - e_c and den per group;
    Ln on ACT; a second matmul with the block-diag class-weight matrix
    (plus -14*ln(den) rows) gives Z[l, n] = row loss if label were l.
  - DVE scalar_tensor_tensor: (labels == iota) * Z, free-dim
    accumulated -> per-core partials [40, 4]; host sums.
"""

import os
import sys

import numpy as np

if "/opt/trn_rl_repo" not in sys.path:
    sys.path.insert(0, "/opt/trn_rl_repo")

import ml_dtypes

B, D, C = 32768, 1024, 10
NCORES = 8
SHARD = B // NCORES  # 4096
NP = 4      # K passes (256 contraction each via DoubleRow)
NG = 4      # col-blocks per stage, repacked to partition groups 10g
NH = 4      # 1024-col stages (pipeline)
QC = SHARD // (NH * NG)  # 256 cols per matmul / group
MID = 5
OPP_W = 2.0
WSCALE = 64.0  # W is scaled by this into fp8; undone in exp's scale

_CACHE: dict = {}


def _build_nc():
    from contextlib import ExitStack

    import concourse.mybir as mybir
    import concourse.tile as tile
    from concourse import bacc

    f32 = mybir.dt.float32
    bf16 = mybir.dt.bfloat16
    fp8 = mybir.dt.float8e4
    Exp = mybir.ActivationFunctionType.Exp
    Ln = mybir.ActivationFunctionType.Ln
    alu = mybir.AluOpType
    DR = mybir.MatmulPerfMode.DoubleRow

    nc = bacc.Bacc(
        "TRN2",
        target_bir_lowering=False,
        debug=False,
        enable_asserts=True,
        num_devices=NCORES,
    )
    # reps_dr[p*128 + k, h*2048 + g*1024 + m] = fp8(reps[core, n, d])
    #   for d = 256p + 128g + k, n = h*1024 + m  (m in [0,1024))
    reps_dr = nc.dram_tensor(
        "reps_dr", [NP * 128, 8192], fp8, kind="ExternalInput"
    ).ap()
    lab40 = nc.dram_tensor("lab40", [40, 1024], f32, kind="ExternalInput").ap()
    # wdr[k, p*32 + g*16 + m] = fp8(W[m, 256p+128g+k] * WSCALE), m<10
    wdr = nc.dram_tensor("wdr", [128, 128], fp8, kind="ExternalInput").ap()
    uzw128 = nc.dram_tensor("uzw128", [128, 44], bf16, kind="ExternalInput").ap()
    wz40 = nc.dram_tensor("wz40", [44, 40], bf16, kind="ExternalInput").ap()
    bias128 = nc.dram_tensor("bias128", [128, 1], f32, kind="ExternalInput").ap()
    iota40 = nc.dram_tensor("iota40", [40, 1], f32, kind="ExternalInput").ap()
    partials = nc.dram_tensor("partials", [40, NH], f32, kind="ExternalOutput").ap()

    with tile.TileContext(nc) as tc:
        with ExitStack() as ctx:
            const_pool = ctx.enter_context(tc.tile_pool(name="const", bufs=1))
            rt_pool = ctx.enter_context(tc.tile_pool(name="rt", bufs=NP))
            e_pool = ctx.enter_context(tc.tile_pool(name="e", bufs=2))
            lnu_pool = ctx.enter_context(tc.tile_pool(name="lnu", bufs=2))
            scr_pool = ctx.enter_context(tc.tile_pool(name="scr", bufs=2))
            lp_pool = ctx.enter_context(
                tc.tile_pool(name="lp", bufs=3, space="PSUM")
            )
            u_pool = ctx.enter_context(tc.tile_pool(name="u", bufs=1, space="PSUM"))
            z_pool = ctx.enter_context(tc.tile_pool(name="z", bufs=1, space="PSUM"))

            # Preload the activation table set that contains BOTH Exp
            # and Ln so the per-stage Exp<->Ln alternation doesn't reload
            # tables (1283 ns each) eight times.
            try:
                from concourse.hw_specs import get_activation_tables

                tabs = list(get_activation_tables(nc.m.arch).items())
                atl_id = next(
                    i
                    for i, (_, funcs) in enumerate(tabs)
                    if Exp in funcs and Ln in funcs
                )
            except Exception:
                atl_id = 6  # natural_log_exp_and_others in act_info.json
            nc.scalar.add_instruction(
                mybir.InstLoadActFuncSet(
                    name=f"I-{nc.next_id()}",
                    act_func_set_id=atl_id,
                    ins=[],
                    outs=[],
                )
            )

            # PE p-state warm-up: ~5 big dummy matmuls ramp the tensor
            # clock (0.65 -> 2.4 GHz after ~3us busy) while DMA streams.
            warm_src = const_pool.tile([128, 512], bf16, tag="warmsrc")
            nc.vector.memset(warm_src[:], 0.0)
            warm = lp_pool.tile([128, 1024], f32, tag="lp", name="warm")
            for _ in range(12):
                nc.tensor.matmul(
                    warm[:, :512], warm_src[:, :128], warm_src[:], start=True, stop=True
                )

            # two persistent e buffers (alternating per stage), memset
            # once so garbage partition rows (32g+10..32g+31) read by the
            # u-matmul hold 0.0 bf16, never NaN (0 * NaN would poison u)
            e_bufs = [
                const_pool.tile([128, QC], bf16, tag=f"ebuf{i}", name=f"ebuf{i}")
                for i in range(2)
            ]
            for t in e_bufs:
                nc.vector.memset(t[:], 0.0)

            acc = const_pool.tile([40, NH], f32, tag="acc")

            rts = []
            for p in range(NP):
                rt = rt_pool.tile([128, 8192], fp8, tag="rt", name=f"rt{p}")
                rts.append(rt)

            # reps stream: all 8 chunks on the single SP HWDGE ring, in
            # stage order. One FIFO queue means completion semaphores fire
            # in a deterministic order, so the Tile scheduler's internal
            # DMA-timing guess can't misorder the static PE queue (with
            # two rings it queued pair1-gated matmuls ahead of the last
            # pair0 ones, stalling PE ~3us). Bandwidth is HBM-limited
    
            # (~300 GB/s/core) either way.
            # pair0 (stages 0/1) as 4 coarse 1MB chunks; stages 2 and 3
            # as 4 x 512KB each so stage-2 compute overlaps stage-3 DMA
            first = True
            for lo, hi in ((0, 4096), (4096, 6144), (6144, 8192)):
                for p in range(NP):
                    # the very first chunk rides the gpsimd SWDGE ring,
                    # whose engine preamble finishes ~1.3us before SP's --
                    # the stream starts that much earlier
                    eng = nc.gpsimd if first else nc.sync
                    first = False
                    eng.dma_start(
                        rts[p][:, lo:hi],
                        reps_dr[p * 128 : (p + 1) * 128, lo:hi],
                    )

            # all consts ride the ACT HWDGE queue: tiny transfers that
            # land by ~13us; keeping them OFF the SP queue matters because
            # each HWDGE ring is FIFO -- anything queued after the 2MB of
            # reps chunks would not transfer until the whole stream drains.
            wdr_t = const_pool.tile([128, 128], fp8, tag="wdr")
            nc.scalar.dma_start(wdr_t[:], wdr)
            uzw_t = const_pool.tile([128, 44], bf16, tag="uzw")
            nc.scalar.dma_start(uzw_t[:], uzw128)
            bias_t = const_pool.tile([10, 1], f32, tag="bias")
            nc.scalar.dma_start(bias_t[:], bias128[:10, :])
            lab_t = const_pool.tile([40, 1024], f32, tag="lab")
            nc.scalar.dma_start(lab_t[:], lab40)
            wz_t = const_pool.tile([44, 40], bf16, tag="wz")
            nc.scalar.dma_start(wz_t[:], wz40)
            iota_t = const_pool.tile([40, 1], f32, tag="iota")
            nc.scalar.dma_start(iota_t[:], iota40)

            # [128, p, g, m16]; lhsT slice is [128, 2, 10] with pair
            # stride 16 (dual-fp8 LDWEIGHTS requires step % 16 == 0)
            wdr_v = wdr_t[:].rearrange("k (p g m) -> k p g m", p=NP, g=2)

            from concourse.tile import add_dep_helper

            mm_insts = {h: [] for h in range(NH)}
            exp_insts = {}
            ln_insts = {}
            umm_insts = {}
            zmm_insts = {}
            stt_insts = {}
            for h in range(NH):
                # logits^T * 64 for this stage: [10, 1024] across 2 banks.
                # One accumulation bracket per bank: start only on the first
                # matmul touching the bank (pending-zero is tracked per
                # 2KB bank row, so later column-blocks inherit the zeroing).
                lp = lp_pool.tile([128, 1024], f32, tag="lp", name=f"lp{h}")
                for p in range(NP):
                    rt_v = rts[p][:].rearrange(
                        "k (hh g m) -> k hh g m", hh=NH, g=2
                    )
                    for cb in range(NG):
                        mm_insts[h].append(
                            nc.tensor.matmul(
                                lp[:C, cb * QC : (cb + 1) * QC],
                                wdr_v[:, p, :, :C],
                                rt_v[:, h, :, cb * QC : (cb + 1) * QC],
                                start=(p == 0 and cb % 2 == 0),
                                stop=(p == NP - 1 and cb % 2 == 1),
                                perf_mode=DR,
                                skip_group_check=True,
                            )
                        )

                # fused repack+exp: ACT reads each psum col-block and
                # writes e at partition offset 32g (offsets must be a
                # multiple of 32): e[32g+j, c] = exp(logit_j + b_j) for
                # col h*1024 + g*256 + c
                e = e_bufs[h % 2]
                exp_insts[h] = [
                    nc.scalar.activation(
                        e[32 * g : 32 * g + C, :],
                        lp[:C, g * QC : (g + 1) * QC],
                        Exp,
                        bias=bias_t[:],
                        scale=1.0 / WSCALE,
                    )
                    for g in range(NG)
                ]

                u = u_pool.tile([128, 512], f32, tag="u", name=f"u{h}")
                umm_insts[h] = nc.tensor.matmul(
                    u[:44, :QC], uzw_t[:], e[:], start=True, stop=True
                )

                lnu = lnu_pool.tile([44, QC], bf16, tag="lnu", name=f"ln{h}")
                ln_insts[h] = nc.scalar.activation(lnu[:], u[:44, :QC], Ln)

                # Z[10g+l, c] = sum_i wmat[i,l]*ln(u_gi) - 14*ln(den_g)
                z = z_pool.tile([128, 512], f32, tag="z", name=f"z{h}")
                zmm_insts[h] = nc.tensor.matmul(
                    z[:40, :QC], wz_t[:], lnu[:], start=True, stop=True
                )

                # partial_l += sum_c (labels == l) * Z[l, c]
                scr = scr_pool.tile([40, QC], f32, tag="scr", name=f"sc{h}")
                stt_insts[h] = nc.vector.scalar_tensor_tensor(
                    out=scr[:],
                    in0=lab_t[:, h * QC : (h + 1) * QC],
                    scalar=iota_t[:],
                    in1=z[:40, :QC],
                    op0=alu.is_equal,
                    op1=alu.mult,
                    accum_out=acc[:, h : h + 1],
                )

            nc.sync.dma_start(partials, acc[:])

            # Pin per-engine instruction order with nosync dep chains --
            # the scheduler's default order interleaves stages and its
            # monotonic completion-counter waits then make early-stage
            # ACT ops wait on late-stage matmuls.
            def chain(insts, why):
                for a, b in zip(insts[1:], insts[:-1]):
                    add_dep_helper(a.ins, b.ins, sync=False, reason=why)

            chain(
                exp_insts[0] + exp_insts[1] + exp_insts[2] + exp_insts[3],
                "ACT exp stage order",
            )
            chain([ln_insts[h] for h in range(NH)], "ACT ln stage order")
            chain([stt_insts[h] for h in range(NH)], "DVE stage order")

    nc.compile()
    return nc


def _prepare_static(W: np.ndarray, b: np.ndarray):
    fp8 = ml_dtypes.float8_e4m3
    bf16 = ml_dtypes.bfloat16

    # wdr[k, p*32 + g*16 + m] = fp8(W[m, 256p + 128g + k] * WSCALE)
    Wt = (W.astype(np.float32).T * WSCALE).reshape(NP, 2, 128, C)
    wdr = np.zeros((128, NP, 2, 16), dtype=np.float32)
    wdr[:, :, :, :C] = Wt.transpose(2, 0, 1, 3)
    wdr = np.ascontiguousarray(wdr).reshape(128, 128).astype(fp8)

    # uzw128: block-diag of [10, 11] blocks (ones - I | ones) at rows 32g
    uzw128 = np.zeros((128, 44), dtype=np.float32)
    blk = np.ones((C, C + 1), dtype=np.float32)
    blk[:, :C] -= np.eye(C, dtype=np.float32)
    for g in range(NG):
        uzw128[32 * g : 32 * g + C, 11 * g : 11 * g + 11] = blk
    uzw128 = uzw128.astype(bf16)  # exact 0/1

    # wz40: block-diag of [11, 10]: wmat (0/1/2) with a -14 den row
    cc = np.arange(C)[:, None]
    ll = np.arange(C)[None, :]
    opp = (cc < MID) != (ll < MID)
    wmat = np.where(cc == ll, 0.0, np.where(opp, OPP_W, 1.0)).astype(np.float32)
    wblk = np.concatenate(
        [wmat, np.full((1, C), -float(C + MID - 1), dtype=np.float32)], axis=0
    )
    wz40 = np.zeros((44, 40), dtype=np.float32)
    for g in range(NG):
        wz40[11 * g : 11 * g + 11, 10 * g : 10 * g + 10] = wblk
    wz40 = wz40.astype(bf16)  # exact small ints

    bias128 = np.zeros((128, 1), dtype=np.float32)
    for g in range(NG):
        bias128[32 * g : 32 * g + C, 0] = b.astype(np.float32)
    iota40 = np.tile(np.arange(C, dtype=np.float32), NG).reshape(40, 1)
    return wdr, uzw128, wz40, bias128, iota40


def kernel(reps, W, b, labels):
    from concourse.bass_utils import run_bass_kernel_spmd

    reps = np.asarray(reps, dtype=np.float32)
    W = np.asarray(W, dtype=np.float32)
    b = np.asarray(b, dtype=np.float32)
    labels_np = np.asarray(labels)

    if "nc" not in _CACHE:
        _CACHE["nc"] = _build_nc()
    nc = _CACHE["nc"]

    wdr, uzw128, wz40, bias128, iota40 = _prepare_static(W, b)

    fp8 = ml_dtypes.float8_e4m3
    reps8 = reps.astype(fp8)  # [B, D]

    in_maps = []
    for core in range(NCORES):
        sh = slice(core * SHARD, (core + 1) * SHARD)
        # [D, SHARD] -> [p, g, k, h, m] -> [p, k, h, g, m] -> [512, 8192]
        shT = reps8[sh].T.reshape(NP, 2, 128, NH, 1024)
        reps_dr = np.ascontiguousarray(shT.transpose(0, 2, 3, 1, 4)).reshape(
            NP * 128, 8192
        )

        lab = labels_np[sh].astype(np.float32).reshape(NH, NG, QC)
        lab40 = np.empty((40, 1024), dtype=np.float32)
        for g in range(NG):
            for h in range(NH):
                lab40[10 * g : 10 * g + C, h * QC : (h + 1) * QC] = lab[h, g][None, :]

        in_maps.append(
            {
                "reps_dr": reps_dr,
                "lab40": lab40,
                "wdr": wdr,
                "uzw128": uzw128,
                "wz40": wz40,
                "bias128": bias128,
                "iota40": iota40,
            }
        )

    trace = bool(int(os.environ.get("CC_KERNEL_TRACE", "0")))
    res = run_bass_kernel_spmd(
        nc, in_maps, core_ids=list(range(NCORES)), trace=trace
    )
    if trace:
        _CACHE["last_results"] = res

    total = np.float64(0.0)
    for core in range(NCORES):
        total += np.float64(res.results[core]["partials"].sum(dtype=np.float64))
    loss = -(total / B)
    return np.float32(loss)


# revision 23
# speedup vs baseline: 1.0186x; 1.0186x over previous
"""Data-parallel Trainium2 kernel for the weighted classification loss.

loss = -mean_b sum_c w[b,c] * log(1 - softmax(reps @ W.T + b)[b,c])

Strategy (8 cores, batch-sharded 4096 rows each):
  - Host pre-transposes and casts reps to fp8 e4m3 (and W, scaled by 64)
    laid out so each DMA chunk is contiguous partition lines; fp8
    quantization error on the final mean loss is ~4e-5 (verified vs f64
    on host).
  - fp8 DoubleRow matmuls (K=256/pass, 4 passes) accumulate logits^T*64
    into PSUM per column-stage. Stage widths are uneven
    (2048/1024/512/512): the later a stage's DMA chunk lands, the less
    post-processing it owes, which shortens the post-stream tail.
  - Fused repack+exp on ACT: each exp reads a psum col-block and writes
    e at partition offset 32g, so downstream ops use 64-128 partitions.
  - One matmul with a block-diag (ones-I | ones) stationary gives
    u = den - e_c and den per group; Ln on ACT; a second matmul with
    the block-diag class-weight matrix (plus -14*ln(den) rows) gives
    Z[l, n] = row loss if the label were l.
  - DVE scalar_tensor_tensor: (labels == iota) * Z, free-dim
    accumulated -> per-core partials [40, 4]; host sums.
  - All reps chunks ride ONE HWDGE ring in stage order (deterministic
    completion order keeps the Tile scheduler's static engine queues
    honest); consts ride the ACT ring; explicit nosync chains pin the
    ACT/DVE instruction order to stage order.
"""

import os
import sys

import numpy as np

if "/opt/trn_rl_repo" not in sys.path:
    sys.path.insert(0, "/opt/trn_rl_repo")

import ml_dtypes

B, D, C = 32768, 1024, 10
NCORES = 8
SHARD = B // NCORES  # 4096
NP = 4      # K passes (256 contraction each via DoubleRow)
NH = 4
# (col_offset, width, n_groups, group_width) per stage
STAGES = [
    (0, 2048, 4, 512),
    (2048, 1024, 4, 256),
    (3072, 512, 2, 256),
    (3584, 512, 2, 256),
]
LABW = sum(s[3] for s in STAGES)  # 1280
LABO = [0]
for s in STAGES[:-1]:
    LABO.append(LABO[-1] + s[3])
MID = 5
OPP_W = 2.0
WSCALE = 64.0  # W is scaled by this into fp8; undone in exp's scale

_CACHE: dict = {}


def _build_nc():
    from contextlib import ExitStack

    import concourse.mybir as mybir
    import concourse.tile as tile
    from concourse import bacc
    from concourse.tile import add_dep_helper

    f32 = mybir.dt.float32
    bf16 = mybir.dt.bfloat16
    fp8 = mybir.dt.float8e4
    Exp = mybir.ActivationFunctionType.Exp
    Ln = mybir.ActivationFunctionType.Ln
    alu = mybir.AluOpType
    DR = mybir.MatmulPerfMode.DoubleRow

    nc = bacc.Bacc(
        "TRN2",
        target_bir_lowering=False,
        debug=False,
        enable_asserts=True,
        num_devices=NCORES,
    )
    # reps_dr[p*128 + k, 2*O + g*W + c] = fp8(reps[core, n, d])
    #   for d = 256p + 128g + k, n = O + c   (per stage (O, W))
    reps_dr = nc.dram_tensor(
        "reps_dr", [NP * 128, 8192], fp8, kind="ExternalInput"
    ).ap()
    lab40 = nc.dram_tensor("lab40", [40, LABW], f32, kind="ExternalInput").ap()
    # wdr[k, p*32 + g*16 + m] = fp8(W[m, 256p+128g+k] * WSCALE), m<10
    wdr = nc.dram_tensor("wdr", [128, 128], fp8, kind="ExternalInput").ap()
    uzw128 = nc.dram_tensor("uzw128", [128, 44], bf16, kind="ExternalInput").ap()
    wz40 = nc.dram_tensor("wz40", [44, 40], bf16, kind="ExternalInput").ap()
    bias128 = nc.dram_tensor("bias128", [128, 1], f32, kind="ExternalInput").ap()
    iota40 = nc.dram_tensor("iota40", [40, 1], f32, kind="ExternalInput").ap()
    partials = nc.dram_tensor("partials", [40, NH], f32, kind="ExternalOutput").ap()

    with tile.TileContext(nc) as tc:
        with ExitStack() as ctx:
            const_pool = ctx.enter_context(tc.tile_pool(name="const", bufs=1))
            rt_pool = ctx.enter_context(tc.tile_pool(name="rt", bufs=NP))
            lnu_pool = ctx.enter_context(tc.tile_pool(name="lnu", bufs=2))
            scr_pool = ctx.enter_context(tc.tile_pool(name="scr", bufs=2))
            lp_pool = ctx.enter_context(
                tc.tile_pool(name="lp", bufs=3, space="PSUM")
            )
            u_pool = ctx.enter_context(tc.tile_pool(name="u", bufs=1, space="PSUM"))
            z_pool = ctx.enter_context(tc.tile_pool(name="z", bufs=1, space="PSUM"))

            # Preload the activation table set that contains BOTH Exp
            # and Ln so the per-stage Exp<->Ln alternation doesn't
            # reload tables (1283 ns each).
            try:
                from concourse.hw_specs import get_activation_tables

                tabs = list(get_activation_tables(nc.m.arch).items())
                atl_id = next(
                    i
                    for i, (_, funcs) in enumerate(tabs)
                    if Exp in funcs and Ln in funcs
                )
            except Exception:
                atl_id = 6  # natural_log_exp_and_others in act_info.json
            nc.scalar.add_instruction(
                mybir.InstLoadActFuncSet(
                    name=f"I-{nc.next_id()}",
                    act_func_set_id=atl_id,
                    ins=[],
                    outs=[],
                )
            )

            # PE p-state warm-up: dummy matmuls ramp the tensor clock
            # (0.65 -> 2.4 GHz after ~3us busy) while DMA streams.
            warm_src = const_pool.tile([128, 512], bf16, tag="warmsrc")
            nc.vector.memset(warm_src[:], 0.0)
            warm = lp_pool.tile([128, 1024], f32, tag="lp", name="warm")
            for _ in range(12):
                nc.tensor.matmul(
                    warm[:, :512], warm_src[:, :128], warm_src[:], start=True, stop=True
                )

            # two persistent e buffers (alternating per stage), memset
            # once so garbage partition rows (32g+10..32g+31) read by the
            # u-matmul hold 0.0 bf16, never NaN (0 * NaN would poison u)
            e_bufs = [
                const_pool.tile([128, 512], bf16, tag=f"ebuf{i}", name=f"ebuf{i}")
                for i in range(2)
            ]
            for t in e_bufs:
                nc.vector.memset(t[:], 0.0)

            acc = const_pool.tile([40, NH], f32, tag="acc")
            nc.vector.memset(acc[:], 0.0)  # stages 2/3 fill rows 0-19 only

            rts = []
            for p in range(NP):
                rt = rt_pool.tile([128, 8192], fp8, tag="rt", name=f"rt{p}")
                rts.append(rt)

            # all consts ride the ACT HWDGE queue: tiny transfers that
            # land early; keeping them OFF the SP queue matters because
            # each HWDGE ring is FIFO -- anything queued after the reps
            # stream would not transfer until it drains.
            wdr_t = const_pool.tile([128, 128], fp8, tag="wdr")
            nc.scalar.dma_start(wdr_t[:], wdr)
            uzw_t = const_pool.tile([128, 44], bf16, tag="uzw")
            nc.scalar.dma_start(uzw_t[:], uzw128)
            bias_t = const_pool.tile([10, 1], f32, tag="bias")
            nc.scalar.dma_start(bias_t[:], bias128[:10, :])
            lab_t = const_pool.tile([40, LABW], f32, tag="lab")
            nc.scalar.dma_start(lab_t[:], lab40)
            wz_t = const_pool.tile([44, 40], bf16, tag="wz")
            nc.scalar.dma_start(wz_t[:], wz40)
            iota_t = const_pool.tile([40, 1], f32, tag="iota")
            nc.scalar.dma_start(iota_t[:], iota40)

            # reps stream: one chunk per (stage, pass) on the single SP
            # HWDGE ring, in stage order. A single FIFO queue means the
            # completion semaphores fire in deterministic order, so the
            # Tile scheduler's internal DMA-timing guess cannot misorder
            # the static PE queue.
            for O, W, _, _ in STAGES:
                for p in range(NP):
                    nc.sync.dma_start(
                        rts[p][:, 2 * O : 2 * O + 2 * W],
                        reps_dr[p * 128 : (p + 1) * 128, 2 * O : 2 * O + 2 * W],
                    )

            # [128, p, g, m16]; lhsT slice is [128, 2, 10] with pair
            # stride 16 (dual-fp8 LDWEIGHTS requires step % 16 == 0)
            wdr_v = wdr_t[:].rearrange("k (p g m) -> k p g m", p=NP, g=2)

            exp_insts = []
            ln_insts = []
            stt_insts = []
            for h, (O, W, NGh, GW) in enumerate(STAGES):
                # psum sub-tiles of <=1024 cols (2 banks); accumulation
                # bracket per 512-col bank: start only on the first
                # matmul touching the bank (pending-zero is tracked per
                # 2KB bank row), stop on the last.
                subs = [(so, min(1024, W - so)) for so in range(0, W, 1024)]
                lps = []
                for si, (so, sw) in enumerate(subs):
                    lp = lp_pool.tile(
                        [128, 1024], f32, tag="lp", name=f"lp{h}_{si}"
                    )
                    lps.append(lp)
                for p in range(NP):
                    view = rts[p][:, 2 * O : 2 * O + 2 * W].rearrange(
                        "k (g m) -> k g m", g=2
                    )
                    for si, (so, sw) in enumerate(subs):
                        for cb in range(sw // 256):
                            cc = so + cb * 256
                            nc.tensor.matmul(
                                lps[si][:C, cb * 256 : (cb + 1) * 256],
                                wdr_v[:, p, :, :C],
                                view[:, :, cc : cc + 256],
                                start=(p == 0 and cb % 2 == 0),
                                stop=(p == NP - 1 and cb % 2 == 1),
                                perf_mode=DR,
                                skip_group_check=True,
                            )

                # fused repack+exp: ACT reads each psum col-block and
                # writes e at partition offset 32g (must be a multiple
                # of 32): e[32g+j, c] = exp(logit_j + b_j)
                e = e_bufs[h % 2]
                for g in range(NGh):
                    si, loc = (g * GW) // 1024, (g * GW) % 1024
                    exp_insts.append(
                        nc.scalar.activation(
                            e[32 * g : 32 * g + C, :GW],
                            lps[si][:C, loc : loc + GW],
                            Exp,
                            bias=bias_t[:],
                            scale=1.0 / WSCALE,
                        )
                    )

                # u rows 11g+i = den_g - e_gi (i<10), row 11g+10 = den_g
                nr = 11 * NGh
                u = u_pool.tile([128, 512], f32, tag="u", name=f"u{h}")
                nc.tensor.matmul(
                    u[:nr, :GW],
                    uzw_t[: 32 * NGh, :nr],
                    e[: 32 * NGh, :GW],
                    start=True,
                    stop=True,
                )

                lnu = lnu_pool.tile([44, 512], bf16, tag="lnu", name=f"ln{h}")
                ln_insts.append(
                    nc.scalar.activation(lnu[:nr, :GW], u[:nr, :GW], Ln)
                )

                # Z[10g+l, c] = sum_i wmat[i,l]*ln(u_gi) - 14*ln(den_g)
                zr = 10 * NGh
                z = z_pool.tile([128, 512], f32, tag="z", name=f"z{h}")
                nc.tensor.matmul(
                    z[:zr, :GW],
                    wz_t[:nr, :zr],
                    lnu[:nr, :GW],
                    start=True,
                    stop=True,
                )

                # partial_l += sum_c (labels == l) * Z[l, c]
                scr = scr_pool.tile([40, 512], f32, tag="scr", name=f"sc{h}")
                stt_insts.append(
                    nc.vector.scalar_tensor_tensor(
                        out=scr[:zr, :GW],
                        in0=lab_t[:zr, LABO[h] : LABO[h] + GW],
                        scalar=iota_t[:zr],
                        in1=z[:zr, :GW],
                        op0=alu.is_equal,
                        op1=alu.mult,
                        accum_out=acc[:zr, h : h + 1],
                    )
                )

            nc.sync.dma_start(partials, acc[:])

            # Pin ACT/DVE instruction order to stage order (the
            # scheduler otherwise interleaves stages and its monotonic
            # completion-counter waits then tie early-stage ACT ops to
            # late-stage matmuls).
            def chain(insts, why):
                for a, b in zip(insts[1:], insts[:-1]):
                    add_dep_helper(a.ins, b.ins, sync=False, reason=why)

            chain(exp_insts, "ACT exp stage order")
            chain(ln_insts, "ACT ln stage order")
            chain(stt_insts, "DVE stage order")

    nc.compile()
    return nc


def _prepare_static(W: np.ndarray, b: np.ndarray):
    fp8 = ml_dtypes.float8_e4m3
    bf16 = ml_dtypes.bfloat16

    # wdr[k, p*32 + g*16 + m] = fp8(W[m, 256p + 128g + k] * WSCALE)
    Wt = (W.astype(np.float32).T * WSCALE).reshape(NP, 2, 128, C)
    wdr = np.zeros((128, NP, 2, 16), dtype=np.float32)
    wdr[:, :, :, :C] = Wt.transpose(2, 0, 1, 3)
    wdr = np.ascontiguousarray(wdr).reshape(128, 128).astype(fp8)

    # uzw128: block-diag of [10, 11] blocks (ones - I | ones) at rows 32g
    uzw128 = np.zeros((128, 44), dtype=np.float32)
    blk = np.ones((C, C + 1), dtype=np.float32)
    blk[:, :C] -= np.eye(C, dtype=np.float32)
    for g in range(4):
        uzw128[32 * g : 32 * g + C, 11 * g : 11 * g + 11] = blk
    uzw128 = uzw128.astype(bf16)  # exact 0/1

    # wz40: block-diag of [11, 10]: wmat (0/1/2) with a -14 den row
    cc = np.arange(C)[:, None]
    ll = np.arange(C)[None, :]
    opp = (cc < MID) != (ll < MID)
    wmat = np.where(cc == ll, 0.0, np.where(opp, OPP_W, 1.0)).astype(np.float32)
    wblk = np.concatenate(
        [wmat, np.full((1, C), -float(C + MID - 1), dtype=np.float32)], axis=0
    )
    wz40 = np.zeros((44, 40), dtype=np.float32)
    for g in range(4):
        wz40[11 * g : 11 * g + 11, 10 * g : 10 * g + 10] = wblk
    wz40 = wz40.astype(bf16)  # exact small ints

    bias128 = np.zeros((128, 1), dtype=np.float32)
    for g in range(4):
        bias128[32 * g : 32 * g + C, 0] = b.astype(np.float32)
    iota40 = np.tile(np.arange(C, dtype=np.float32), 4).reshape(40, 1)
    return wdr, uzw128, wz40, bias128, iota40


def kernel(reps, W, b, labels):
    from concourse.bass_utils import run_bass_kernel_spmd

    reps = np.asarray(reps, dtype=np.float32)
    W = np.asarray(W, dtype=np.float32)
    b = np.asarray(b, dtype=np.float32)
    labels_np = np.asarray(labels)

    if "nc" not in _CACHE:
        _CACHE["nc"] = _build_nc()
    nc = _CACHE["nc"]

    wdr, uzw128, wz40, bias128, iota40 = _prepare_static(W, b)

    fp8 = ml_dtypes.float8_e4m3
    reps8 = reps.astype(fp8)  # [B, D]

    in_maps = []
    for core in range(NCORES):
        sh = slice(core * SHARD, (core + 1) * SHARD)
        # [D, SHARD] as [p][g][k][c]; per stage slice cols and lay out
        # [p][k][g][W]; concat stages along the free axis
        R = reps8[sh].T.reshape(NP, 2, 128, SHARD)
        parts = [
            np.ascontiguousarray(R[:, :, :, O : O + W].transpose(0, 2, 1, 3)).reshape(
                NP, 128, 2 * W
            )
            for O, W, _, _ in STAGES
        ]
        reps_dr = np.concatenate(parts, axis=2).reshape(NP * 128, 8192)

        lab = labels_np[sh].astype(np.float32)
        lab40 = np.zeros((40, LABW), dtype=np.float32)
        for h, (O, W, NGh, GW) in enumerate(STAGES):
            for g in range(NGh):
                seg = lab[O + g * GW : O + (g + 1) * GW]
                lab40[10 * g : 10 * g + C, LABO[h] : LABO[h] + GW] = seg[None, :]

        in_maps.append(
            {
                "reps_dr": reps_dr,
                "lab40": lab40,
                "wdr": wdr,
                "uzw128": uzw128,
                "wz40": wz40,
                "bias128": bias128,
                "iota40": iota40,
            }
        )

    trace = bool(int(os.environ.get("CC_KERNEL_TRACE", "0")))
    res = run_bass_kernel_spmd(
        nc, in_maps, core_ids=list(range(NCORES)), trace=trace
    )
    if trace:
        _CACHE["last_results"] = res

    total = np.float64(0.0)
    for core in range(NCORES):
        total += np.float64(res.results[core]["partials"].sum(dtype=np.float64))
    loss = -(total / B)
    return np.float32(loss)
